# revision 1
# baseline (speedup 1.0000x reference)
"""CapsuleLayer (dynamic routing, ROUTING_ITER=2) Bass/Tile kernel for TRN2.

Contract: kernel(x, weight) takes FULL inputs
  x:      [64, 2048, 1, 16] f32
  weight: [1, 2048, 32, 16, 16] f32
returns FULL output [64, 32, 16] f32.

Sharding: data-parallel over batch B=64 across 8 cores (8 per core),
weight replicated. Self-contained: hardcodes shapes, imports only
numpy/ml_dtypes/concourse.
"""

from contextlib import ExitStack

import ml_dtypes
import numpy as np

import concourse.bacc as bacc
import concourse.bass as bass
import concourse.mybir as mybir
import concourse.tile as tile
from concourse.bass_utils import run_bass_kernel_spmd

F32 = mybir.dt.float32
BF16 = mybir.dt.bfloat16
AF = mybir.ActivationFunctionType
AX = mybir.AxisListType

EPS = 1e-8
J, D, E = 32, 16, 16
JD = J * D  # 512


def emit_capsule(tc, w2, xbd, d1, ds, out, n_in, b_loc=8):
    """Emit the per-core capsule program.

    DRAM tensors (APs):
      w2  [G, 8, E, JD] bf16  weight, host-permuted to [i, e, d, j], i=g*8+di
      xbd [G, 128, 64]  bf16  block-diag x stationary: [g, (di,e), (b,di')]
      d1  [128, 8] bf16       delta matrix * (1/32)  (s1 accumulation)
      ds  [128, 8] bf16       delta matrix * 1.0     (s2/s3 accumulation)
      out [b_loc, JD] f32     squash(s3) output, (j,d) layout

    u layout in SBUF (bf16): partition p = (g%2)*64 + b*8 + di,
    free f = (g//2)*JD + d*J + j, where i = g*8 + di.
    """
    nc = tc.nc
    assert b_loc == 8
    G = n_in // 8
    GH = G // 2
    GDMA = 16  # groups per W DMA chunk
    CH = 8    # gh per routing chunk
    assert G % GDMA == 0 and GH % CH == 0
    NCH = GH // CH

    ctx = ExitStack()
    singles = ctx.enter_context(tc.tile_pool(name="singles", bufs=1))
    small = ctx.enter_context(tc.tile_pool(name="small", bufs=2))
    dramp = ctx.enter_context(tc.tile_pool(name="dram_scratch", bufs=1, space="DRAM"))

    u_sb = singles.tile([128, GH * JD], BF16)
    d1_sb = singles.tile([128, 8], BF16)
    ds_sb = singles.tile([128, 8], BF16)
    nc.sync.dma_start(out=d1_sb, in_=d1)
    nc.sync.dma_start(out=ds_sb, in_=ds)
    v_exp = singles.tile([128, JD], BF16)
    V = singles.tile([8, JD], F32)      # running sum of v vectors
    s_sb = singles.tile([8, JD], F32)
    vscr = dramp.tile([8, JD], BF16)    # DRAM bounce buffer for v broadcast
    eps8 = singles.tile([8, 1], F32)
    nc.vector.memset(eps8, EPS)

    # ---------- squash helpers (all on 8 partitions, tiny) ----------
    def squash_j(s_in, v_out):
        # v = squash(s, axis=j):  sq[b,d] = sum_j s^2;  v = s*sq/((1+sq)*sqrt(sq+eps))
        t2 = small.tile([8, JD], F32, tag="sqt2")
        nc.vector.tensor_mul(t2, s_in, s_in)
        sv = small.tile([8, 4, J], F32, tag="sqv")
        sq, a, t3, w = sv[:, 0, :D], sv[:, 1, :D], sv[:, 2, :D], sv[:, 3, :D]
        nc.vector.reduce_sum(out=sq, in_=t2.rearrange("p (d j) -> p d j", d=D), axis=AX.X)
        nc.scalar.activation(a, sq, AF.Sqrt, bias=eps8)
        nc.vector.tensor_mul(t3, sq, a)
        nc.vector.tensor_add(t3, t3, a)          # a*(1+sq)
        nc.vector.reciprocal(w, t3)
        nc.vector.tensor_mul(w, w, sq)           # sq/((1+sq)a)
        wb = w.unsqueeze(2).broadcast_to([8, D, J])
        nc.vector.tensor_mul(v_out.rearrange("p (d j) -> p d j", d=D),
                             s_in.rearrange("p (d j) -> p d j", d=D), wb)

    def squash_d(s_in, v_out):
        # v = squash(s, axis=d): sq[b,j] = sum_d s^2
        t2 = small.tile([8, JD], F32, tag="sqt2")
        nc.vector.tensor_mul(t2, s_in, s_in)
        sv = small.tile([8, 4, J], F32, tag="sqv")
        sq, a, t3, w = sv[:, 0, :], sv[:, 1, :], sv[:, 2, :], sv[:, 3, :]
        nc.vector.reduce_sum(out=sq, in_=t2.rearrange("p (d j) -> p j d", d=D), axis=AX.X)
        nc.scalar.activation(a, sq, AF.Sqrt, bias=eps8)
        nc.vector.tensor_mul(t3, sq, a)
        nc.vector.tensor_add(t3, t3, a)
        nc.vector.reciprocal(w, t3)
        nc.vector.tensor_mul(w, w, sq)
        wb = w.unsqueeze(1).broadcast_to([8, D, J])
        nc.vector.tensor_mul(v_out.rearrange("p (d j) -> p d j", d=D),
                             s_in.rearrange("p (d j) -> p d j", d=D), wb)

    def refresh_v_exp():
        vb = small.tile([8, JD], BF16, tag="vb")
        nc.vector.tensor_copy(out=vb, in_=V)
        nc.sync.dma_start(out=vscr, in_=vb)
        src = vscr.unsqueeze(1).broadcast_to([8, 8, JD])
        for g2 in range(2):
            nc.sync.dma_start(out=v_exp[g2 * 64:(g2 + 1) * 64, :], in_=src)

    # ---------- phase 1: W pass (u = W @ x), s1 accumulation ----------
    with tc.tile_pool(name="wp", bufs=3) as wp, \
         tc.tile_pool(name="xp", bufs=2) as xp, \
         tc.tile_pool(name="up", bufs=3, space="PSUM") as up, \
         tc.tile_pool(name="sp", bufs=1, space="PSUM") as sp:
        s1_ps = sp.tile([8, JD], F32)
        for ci in range(G // GDMA):
            wt = wp.tile([128, GDMA, JD], BF16, tag="wt")
            # src: dims (k=(di,e) merged, g, jd)
            wsrc = w2[ci * GDMA:(ci + 1) * GDMA].rearrange("g di e f -> (di e) g f")
            nc.sync.dma_start(out=wt, in_=wsrc)
            xt = xp.tile([128, GDMA, 64], BF16, tag="xt")
            xsrc = xbd[ci * GDMA:(ci + 1) * GDMA].rearrange("g k m -> k g m")
            nc.sync.dma_start(out=xt, in_=xsrc)
            for gq in range(GDMA // 4):
                pt = up.tile([128, 2 * JD], F32, tag="upt")
                for idx in range(4):
                    gl = gq * 4 + idx
                    nc.tensor.matmul(
                        pt[(gl % 2) * 64:(gl % 2) * 64 + 64,
                           (idx // 2) * JD:(idx // 2) * JD + JD],
                        xt[:, gl, :], wt[:, gl, :], start=True, stop=True)
                gh0 = (ci * GDMA) // 2 + gq * 2
                if gq % 2 == 0:
                    nc.vector.tensor_copy(out=u_sb[:, gh0 * JD:(gh0 + 2) * JD], in_=pt)
                else:
                    nc.scalar.copy(out=u_sb[:, gh0 * JD:(gh0 + 2) * JD], in_=pt)
                for gh in (gh0, gh0 + 1):
                    nc.tensor.matmul(s1_ps, d1_sb, u_sb[:, gh * JD:(gh + 1) * JD],
                                     start=(gh == 0), stop=(gh == GH - 1))
        nc.vector.tensor_copy(out=s_sb, in_=s1_ps)

    squash_j(s_sb, V)      # V = v1
    refresh_v_exp()

    # ---------- routing pass (T = u.V, softmax, s = sum_i c*u) ----------
    def routing_pass(final):
        with tc.tile_pool(name="rp1", bufs=2) as rp1, \
             tc.tile_pool(name="rp", bufs=3) as rp, \
             tc.tile_pool(name="yp", bufs=2) as yp, \
             tc.tile_pool(name="spp", bufs=1, space="PSUM") as spp:
            s_ps = spp.tile([8, JD], F32)
            for k in range(NCH):
                gh0 = k * CH
                fs, fe = gh0 * JD, (gh0 + CH) * JD
                u_ch = u_sb[:, fs:fe].rearrange("p (g f) -> p g f", g=CH)
                def prod4_u(uc):
                    return uc.rearrange("p g (d j) -> p g d j", d=D)
                prod = rp1.tile([128, CH, JD], BF16, tag="prod")
                vb = v_exp.unsqueeze(1).broadcast_to([128, CH, JD])
                nc.vector.tensor_mul(prod, u_ch, vb)
                # tree-reduce over d (outer dim: 16 -> 8 -> 4 -> 2 -> 1), all bf16
                p4 = prod.rearrange("p g (d j) -> p g d j", d=D)
                t1 = rp1.tile([128, CH, 8, J], BF16, tag="t1")
                nc.vector.tensor_add(t1, p4[:, :, 0:8, :], p4[:, :, 8:16, :])
                t2 = rp1.tile([128, CH, 4, J], BF16, tag="t2")
                nc.vector.tensor_add(t2, t1[:, :, 0:4, :], t1[:, :, 4:8, :])
                t3 = rp1.tile([128, CH, 2, J], BF16, tag="t3")
                nc.vector.tensor_add(t3, t2[:, :, 0:2, :], t2[:, :, 2:4, :])
                tt = rp.tile([128, CH, J], BF16, tag="tt")
                nc.vector.tensor_add(tt.unsqueeze(2),
                                     t3[:, :, 0:1, :], t3[:, :, 1:2, :])
                # softmax over j (no max subtraction; logits are tiny)
                eT = rp.tile([128, CH, J], BF16, tag="eT")
                nc.scalar.activation(eT, tt, AF.Exp)
                se = rp.tile([128, CH], F32, tag="se")
                nc.vector.reduce_sum(out=se, in_=eT, axis=AX.X)
                r = rp.tile([128, CH], F32, tag="r")
                nc.vector.reciprocal(r, se)
                c = rp.tile([128, CH, J], BF16, tag="c")
                nc.vector.tensor_mul(c, eT, r.unsqueeze(2).broadcast_to([128, CH, J]))
                # y = c (broadcast over d, an outer step-0 dim) * u
                y = yp.tile([128, CH, D, J], BF16, tag="y")
                cb = c.unsqueeze(2).broadcast_to([128, CH, D, J])
                nc.vector.tensor_mul(y, prod4_u(u_ch), cb)
                for q in range(CH):
                    gh = gh0 + q
                    nc.tensor.matmul(s_ps, ds_sb, y[:, q].rearrange("p d j -> p (d j)"),
                                     start=(gh == 0), stop=(gh == GH - 1))
            nc.vector.tensor_copy(out=s_sb, in_=s_ps)
        if not final:
            v2 = small.tile([8, JD], F32, tag="v2")
            squash_j(s_sb, v2)
            nc.vector.tensor_add(V, V, v2)
            refresh_v_exp()
        else:
            vout = small.tile([8, JD], F32, tag="vout")
            squash_d(s_sb, vout)
            nc.sync.dma_start(out=out, in_=vout)

    routing_pass(final=False)   # iteration 2 (uses V=v1)
    routing_pass(final=True)    # final (uses V=v1+v2)
    ctx.close()


def build_module(n_in=2048, b_loc=8, num_devices=8, enable_asserts=False):
    nc = bacc.Bacc("TRN2", target_bir_lowering=False, debug=False,
                   num_devices=num_devices, enable_asserts=enable_asserts)
    G = n_in // 8
    w2 = nc.dram_tensor("w2", [G, 8, E, JD], BF16, kind="ExternalInput").ap()
    xbd = nc.dram_tensor("xbd", [G, 128, 64], BF16, kind="ExternalInput").ap()
    d1 = nc.dram_tensor("d1", [128, 8], BF16, kind="ExternalInput").ap()
    ds = nc.dram_tensor("ds", [128, 8], BF16, kind="ExternalInput").ap()
    out = nc.dram_tensor("out", [b_loc, JD], F32, kind="ExternalOutput").ap()
    with tile.TileContext(nc) as tc:
        emit_capsule(tc, w2, xbd, d1, ds, out, n_in=n_in, b_loc=b_loc)
    nc.compile()
    return nc


def host_prep_w(weight, n_in):
    # weight [1, N, J, D, E] -> w2 [G, 8, E, J*D] with free layout (d, j)
    w2 = np.ascontiguousarray(weight[0].transpose(0, 3, 2, 1))  # [N, E, D, J]
    return w2.reshape(n_in // 8, 8, E, JD).astype(ml_dtypes.bfloat16)


def host_prep_xbd(xs, n_in):
    # xs [b_loc, N, E] -> xbd [G, 128, 64] block-diagonal stationary
    G = n_in // 8
    t = xs.reshape(8, G, 8, E).transpose(1, 2, 3, 0)  # [G, di, e, b]
    xbd = np.zeros((G, 8, E, 8, 8), np.float32)       # [G, di, e, b, di']
    for di in range(8):
        xbd[:, di, :, :, di] = t[:, di]
    return xbd.reshape(G, 128, 64).astype(ml_dtypes.bfloat16)


def host_prep_deltas():
    p = np.arange(128)
    bofp = (p // 8) % 8
    d1 = np.zeros((128, 8), np.float32)
    ds = np.zeros((128, 8), np.float32)
    d1[p, bofp] = 1.0 / 32.0
    ds[p, bofp] = 1.0
    return d1.astype(ml_dtypes.bfloat16), ds.astype(ml_dtypes.bfloat16)


_CACHE = {}
LAST_EXEC_NS = None


def kernel(x, weight, trace=False):
    B, N_in = 64, 2048
    n_cores = 8
    b_loc = B // n_cores
    key = (N_in, b_loc, n_cores)
    if key not in _CACHE:
        _CACHE[key] = build_module(n_in=N_in, b_loc=b_loc, num_devices=n_cores)
    nc = _CACHE[key]

    x = np.asarray(x, dtype=np.float32)
    weight = np.asarray(weight, dtype=np.float32)
    w2 = host_prep_w(weight, N_in)
    d1, ds = host_prep_deltas()
    in_maps = []
    for c in range(n_cores):
        xs = np.ascontiguousarray(x[c * b_loc:(c + 1) * b_loc, :, 0, :])
        in_maps.append({
            "w2": w2,
            "xbd": host_prep_xbd(xs, N_in),
            "d1": d1,
            "ds": ds,
        })
    global LAST_EXEC_NS
    res = run_bass_kernel_spmd(nc, in_maps, core_ids=list(range(n_cores)),
                               trace=trace)
    LAST_EXEC_NS = res.exec_time_ns
    outs = [r["out"].reshape(b_loc, D, J).transpose(0, 2, 1) for r in res.results]
    return np.ascontiguousarray(np.concatenate(outs, axis=0))



# revision 2
# speedup vs baseline: 1.0170x; 1.0170x over previous
"""CapsuleLayer (dynamic routing, ROUTING_ITER=2) Bass/Tile kernel for TRN2 — v2.

Contract: kernel(x, weight) takes FULL inputs
  x:      [64, 2048, 1, 16] f32
  weight: [1, 2048, 32, 16, 16] f32
returns FULL output [64, 32, 16] f32.

Sharding: data-parallel over batch B=64 across 8 cores (8 per core),
weight replicated. Self-contained: hardcodes shapes, imports only
numpy/ml_dtypes/concourse.

v2 changes vs v1:
 - T = sum_d(u*v) reduction moved from a DVE bf16 add-tree to PE
   identity-stationary matmuls accumulating in PSUM (f32).
 - prod/y elementwise multiplies split DVE / GPSIMD(Pool).
 - softmax: exp on Act engine straight out of PSUM; sum/recip/scale DVE.
 - xbd host layout transposed so DMA descriptors are 2KB contiguous.
"""

from contextlib import ExitStack

import ml_dtypes
import numpy as np

import concourse.bacc as bacc
import concourse.bass as bass
import concourse.mybir as mybir
import concourse.tile as tile
from concourse.bass_utils import run_bass_kernel_spmd

F32 = mybir.dt.float32
BF16 = mybir.dt.bfloat16
AF = mybir.ActivationFunctionType
AX = mybir.AxisListType

EPS = 1e-8
USE_GPSIMD = True
J, D, E = 32, 16, 16
JD = J * D  # 512


def emit_capsule(tc, w2, xbd, d1, ds, ident, out, n_in, b_loc=8):
    """Emit the per-core capsule program.

    DRAM tensors (APs):
      w2    [G, 8, E, JD] bf16  weight, host-permuted to [i, e, d, j], i=g*8+di
      xbd   [128, G, 64]  bf16  block-diag x stationary: [(di,e), g, (b,di')]
      d1    [128, 8] bf16       delta matrix * (1/32)  (s1 accumulation)
      ds    [128, 8] bf16       delta matrix * 1.0     (s2/s3 accumulation)
      ident [128, 128] bf16     identity (PE partition-copy stationary)
      out   [b_loc, JD] f32     squash(s3) output, (j,d) layout

    u layout in SBUF (bf16): partition p = (g%2)*64 + b*8 + di,
    free f = (g//2)*JD + d*J + j, where i = g*8 + di.
    """
    nc = tc.nc
    assert b_loc == 8
    G = n_in // 8
    GH = G // 2          # 128 gh slices of [128, JD]
    GDMA = 16            # groups per W DMA chunk
    CHB = 16             # gh per routing chunk (16*JD = 8K free)
    NCH = GH // CHB      # 8 chunks per pass
    # gh split per chunk between DVE and GPSIMD for prod/y muls
    GH_DVE = 12

    ctx = ExitStack()
    singles = ctx.enter_context(tc.tile_pool(name="singles", bufs=1))
    small = ctx.enter_context(tc.tile_pool(name="small", bufs=1))
    dramp = ctx.enter_context(tc.tile_pool(name="dram_scratch", bufs=1, space="DRAM"))

    u_sb = singles.tile([128, GH * JD], BF16)
    d1_sb = singles.tile([128, 8], BF16)
    ds_sb = singles.tile([128, 8], BF16)
    id_sb = singles.tile([128, 128], BF16)
    nc.sync.dma_start(out=d1_sb, in_=d1)
    nc.sync.dma_start(out=ds_sb, in_=ds)
    nc.sync.dma_start(out=id_sb, in_=ident)
    v_exp = singles.tile([128, JD], BF16)
    V = singles.tile([8, JD], F32)      # running sum of v vectors
    s_sb = singles.tile([8, JD], F32)
    vscr = dramp.tile([8, JD], BF16)    # DRAM bounce buffer for v broadcast
    eps8 = singles.tile([8, 1], F32)
    nc.vector.memset(eps8, EPS)
    warmp = ctx.enter_context(tc.tile_pool(name="warm", bufs=1, space="PSUM"))
    warm_ps = warmp.tile([128, JD], F32)

    def pe_warm(n=24, src=None):
        # throwaway matmuls that keep the PE clock ramped across the serial
        # squash/v-broadcast boundaries (it would otherwise idle and restart
        # its p-state ramp at the start of the next pass). Passing src chains
        # them on a boundary-produced tile so they fill the boundary itself.
        mov = u_sb[:, :JD]
        for _ in range(n):
            nc.tensor.matmul(warm_ps, id_sb, mov, start=True, stop=True)

    # ---------- squash helpers (all on 8 partitions, tiny) ----------
    def squash_j(s_in, v_out):
        # v = squash(s, axis=j):  sq[b,d] = sum_j s^2;  v = s*sq/((1+sq)*sqrt(sq+eps))
        t2 = small.tile([8, JD], F32, tag="sqt2")
        nc.scalar.square(t2, s_in)
        sv = small.tile([8, 4, J], F32, tag="sqv")
        sq, a, t3, w = sv[:, 0, :D], sv[:, 1, :D], sv[:, 2, :D], sv[:, 3, :D]
        nc.vector.reduce_sum(out=sq, in_=t2.rearrange("p (d j) -> p d j", d=D), axis=AX.X)
        nc.scalar.activation(a, sq, AF.Sqrt, bias=eps8)
        nc.vector.tensor_mul(t3, sq, a)
        nc.vector.tensor_add(t3, t3, a)          # a*(1+sq)
        nc.vector.reciprocal(w, t3)
        nc.vector.tensor_mul(w, w, sq)           # sq/((1+sq)a)
        wb = w.unsqueeze(2).broadcast_to([8, D, J])
        nc.vector.tensor_mul(v_out.rearrange("p (d j) -> p d j", d=D),
                             s_in.rearrange("p (d j) -> p d j", d=D), wb)

    def squash_d(s_in, v_out):
        # v = squash(s, axis=d): sq[b,j] = sum_d s^2
        t2 = small.tile([8, JD], F32, tag="sqt2")
        nc.scalar.square(t2, s_in)
        sv = small.tile([8, 4, J], F32, tag="sqv")
        sq, a, t3, w = sv[:, 0, :], sv[:, 1, :], sv[:, 2, :], sv[:, 3, :]
        nc.vector.reduce_sum(out=sq, in_=t2.rearrange("p (d j) -> p j d", d=D), axis=AX.X)
        nc.scalar.activation(a, sq, AF.Sqrt, bias=eps8)
        nc.vector.tensor_mul(t3, sq, a)
        nc.vector.tensor_add(t3, t3, a)
        nc.vector.reciprocal(w, t3)
        nc.vector.tensor_mul(w, w, sq)
        wb = w.unsqueeze(1).broadcast_to([8, D, J])
        nc.vector.tensor_mul(v_out.rearrange("p (d j) -> p d j", d=D),
                             s_in.rearrange("p (d j) -> p d j", d=D), wb)

    def refresh_v_exp():
        vb = small.tile([8, JD], BF16, tag="vb")
        nc.vector.tensor_copy(out=vb, in_=V)
        nc.sync.dma_start(out=vscr, in_=vb)
        src = vscr.unsqueeze(1).broadcast_to([8, 8, JD])
        for g2 in range(2):
            nc.sync.dma_start(out=v_exp[g2 * 64:(g2 + 1) * 64, :], in_=src)
        return vb

    # ---------- phase 1: W pass (u = W @ x), s1 accumulation ----------
    with tc.tile_pool(name="wp", bufs=3) as wp, \
         tc.tile_pool(name="xp", bufs=2) as xp, \
         tc.tile_pool(name="up", bufs=2, space="PSUM") as up, \
         tc.tile_pool(name="sp", bufs=1, space="PSUM") as sp:
        s1_ps = sp.tile([8, JD], F32)
        # first W chunks are small so PE starts ~7us earlier; rest are 16-group
        chunks = [4, 4, 4, 4] + [GDMA] * ((G - 16) // GDMA)
        g_base = 0
        for csz in chunks:
            g0, g1 = g_base, g_base + csz
            g_base = g1
            wt = wp.tile([128, csz, JD], BF16, tag=f"wt{csz}",
                         bufs=2 if csz < GDMA else None)
            # src: dims (k=(di,e) merged, g, jd)
            wsrc = w2[g0:g1].rearrange("g di e f -> (di e) g f")
            nc.sync.dma_start(out=wt, in_=wsrc)
            xt = xp.tile([128, csz, 64], BF16, tag=f"xt{csz}")
            nc.sync.dma_start(out=xt, in_=xbd[:, g0:g1])
            for gq in range(csz // 4):
                pt = up.tile([128, 2 * JD], F32, tag="upt")
                for idx in range(4):
                    gl = gq * 4 + idx
                    nc.tensor.matmul(
                        pt[(gl % 2) * 64:(gl % 2) * 64 + 64,
                           (idx // 2) * JD:(idx // 2) * JD + JD],
                        xt[:, gl, :], wt[:, gl, :], start=True, stop=True)
                gh0 = g0 // 2 + gq * 2
                dst = u_sb[:, gh0 * JD:(gh0 + 2) * JD]
                if gq % 2 == 0:
                    nc.vector.tensor_copy(out=dst, in_=pt)
                else:
                    nc.scalar.copy(out=dst, in_=pt)
                for gh in (gh0, gh0 + 1):
                    nc.tensor.matmul(s1_ps, d1_sb, u_sb[:, gh * JD:(gh + 1) * JD],
                                     start=(gh == 0), stop=(gh == GH - 1))
        squash_j(s1_ps, V)      # V = v1
        vb = refresh_v_exp()
        pe_warm(40, src=vb)

    # ---------- routing pass (T = u.V via PE, softmax, s = sum_i c*u) ----------
    def routing_pass(final):
        # software-pipelined: stage A(k) = prod + T matmuls; stage B(k) =
        # softmax + y + s matmuls, issued one chunk behind so DVE/Pool never
        # stall waiting on PE/Act.
        with tc.tile_pool(name="rp", bufs=3) as rp, \
             tc.tile_pool(name="ep", bufs=2) as epool, \
             tc.tile_pool(name="cp", bufs=2) as cp, \
             tc.tile_pool(name="tp", bufs=2, space="PSUM") as tp, \
             tc.tile_pool(name="spp", bufs=1, space="PSUM") as spp:
            s_ps = spp.tile([8, JD], F32)
            tiles = {}

            def stage_prod(k):
                gh0 = k * CHB
                u_ch = u_sb[:, gh0 * JD:(gh0 + CHB) * JD].rearrange(
                    "p (g f) -> p g f", g=CHB)
                vb = v_exp.unsqueeze(1).broadcast_to([128, CHB, JD])
                prod = rp.tile([128, CHB, JD], BF16, tag="prod")
                # prod = u * v_bcast, split DVE(13) / GPSIMD(3) by gh range
                if USE_GPSIMD:
                    nc.vector.tensor_mul(prod[:, :13], u_ch[:, :13], vb[:, :13])
                    nc.gpsimd.tensor_mul(prod[:, 13:], u_ch[:, 13:], vb[:, 13:])
                else:
                    nc.vector.tensor_mul(prod, u_ch, vb)
                tiles[k] = (u_ch, prod)

            def stage_T(k):
                u_ch, prod = tiles[k]
                # T[p, gh, j] = sum_d prod[p, gh, d, j]  on PE via identity
                p4 = prod.rearrange("p g (d j) -> p g d j", d=D)
                t_ps = tp.tile([128, CHB, J], F32, tag="T")
                for d in range(D):
                    nc.tensor.matmul(t_ps, id_sb, p4[:, :, d, :],
                                     start=(d == 0), stop=(d == D - 1))
                # softmax over j (no max subtraction; logits are tiny)
                eT = epool.tile([128, CHB, J], BF16, tag="eT")
                nc.scalar.activation(eT, t_ps, AF.Exp)
                tiles[k] = (u_ch, prod, p4, eT)

            def stage_y(k):
                u_ch, prod, p4, eT = tiles[k]
                u4 = u_ch.rearrange("p g (d j) -> p g d j", d=D)
                se = cp.tile([128, CHB], F32, tag="se")
                nc.vector.reduce_sum(out=se, in_=eT, axis=AX.X)
                r = cp.tile([128, CHB], BF16, tag="r")
                with nc.allow_low_precision(reason="softmax denom, rel err 4e-3 ok"):
                    nc.vector.reciprocal(r, se)
                c = cp.tile([128, CHB, J], BF16, tag="c")
                nc.vector.tensor_mul(c, eT, r.unsqueeze(2).broadcast_to([128, CHB, J]))
                # y = c (broadcast over d) * u — overwrites prod in place,
                # DVE(12) / GPSIMD(4) split
                cb = c.unsqueeze(2).broadcast_to([128, CHB, D, J])
                if USE_GPSIMD:
                    nc.vector.tensor_mul(p4[:, :12], u4[:, :12], cb[:, :12])
                    nc.gpsimd.tensor_mul(p4[:, 12:], u4[:, 12:], cb[:, 12:])
                else:
                    nc.vector.tensor_mul(p4, u4, cb)

            def stage_s(k):
                gh0 = k * CHB
                _, prod, _, _ = tiles.pop(k)
                for q in range(CHB):
                    gh = gh0 + q
                    nc.tensor.matmul(s_ps, ds_sb, prod[:, q],
                                     start=(gh == 0), stop=(gh == GH - 1))

            # PE order per iteration: s(k-2) [deps long ready], filler warm
            # matmuls, then T(k) — backlog covers prod(k) latency so the PE
            # stream (and its p-state ramp) never breaks
            for k in range(NCH + 2):
                if k < NCH:
                    stage_prod(k)
                if 1 <= k <= NCH:
                    stage_y(k - 1)
                if k >= 2:
                    stage_s(k - 2)
                if k < NCH:
                    if k >= 1:
                        pe_warm(4)
                    stage_T(k)
            if not final:
                v2 = small.tile([8, JD], F32, tag="v2")
                squash_j(s_ps, v2)
                nc.vector.tensor_add(V, V, v2)
                vb = refresh_v_exp()
                pe_warm(40, src=vb)
            else:
                vout = small.tile([8, JD], F32, tag="vout")
                squash_d(s_ps, vout)
                nc.sync.dma_start(out=out, in_=vout)

    routing_pass(final=False)   # iteration 2 (uses V=v1)
    routing_pass(final=True)    # final (uses V=v1+v2)
    ctx.close()


def build_module(n_in=2048, b_loc=8, num_devices=8, enable_asserts=False):
    nc = bacc.Bacc("TRN2", target_bir_lowering=False, debug=False,
                   num_devices=num_devices, enable_asserts=enable_asserts)
    G = n_in // 8
    w2 = nc.dram_tensor("w2", [G, 8, E, JD], BF16, kind="ExternalInput").ap()
    xbd = nc.dram_tensor("xbd", [128, G, 64], BF16, kind="ExternalInput").ap()
    d1 = nc.dram_tensor("d1", [128, 8], BF16, kind="ExternalInput").ap()
    ds = nc.dram_tensor("ds", [128, 8], BF16, kind="ExternalInput").ap()
    ident = nc.dram_tensor("ident", [128, 128], BF16, kind="ExternalInput").ap()
    out = nc.dram_tensor("out", [b_loc, JD], F32, kind="ExternalOutput").ap()
    with tile.TileContext(nc) as tc:
        emit_capsule(tc, w2, xbd, d1, ds, ident, out, n_in=n_in, b_loc=b_loc)
    nc.compile()
    return nc


def host_prep_w(weight, n_in):
    # weight [1, N, J, D, E] -> w2 [G, 8, E, J*D] with free layout (d, j)
    w2 = np.ascontiguousarray(weight[0].transpose(0, 3, 2, 1))  # [N, E, D, J]
    return w2.reshape(n_in // 8, 8, E, JD).astype(ml_dtypes.bfloat16)


def host_prep_xbd(xs, n_in):
    # xs [b_loc, N, E] -> xbd [128, G, 64] block-diagonal stationary,
    # partition-major so each DMA descriptor is one contiguous 2KB run.
    G = n_in // 8
    t = xs.reshape(8, G, 8, E).transpose(1, 2, 3, 0)  # [G, di, e, b]
    xbd = np.zeros((G, 8, E, 8, 8), np.float32)       # [G, di, e, b, di']
    for di in range(8):
        xbd[:, di, :, :, di] = t[:, di]
    xbd = xbd.reshape(G, 128, 64).transpose(1, 0, 2)  # [(di,e), G, (b,di')]
    return np.ascontiguousarray(xbd).astype(ml_dtypes.bfloat16)


def host_prep_deltas():
    p = np.arange(128)
    bofp = (p // 8) % 8
    d1 = np.zeros((128, 8), np.float32)
    ds = np.zeros((128, 8), np.float32)
    d1[p, bofp] = 1.0 / 32.0
    ds[p, bofp] = 1.0
    return d1.astype(ml_dtypes.bfloat16), ds.astype(ml_dtypes.bfloat16)


_CACHE = {}
LAST_EXEC_NS = None


def kernel(x, weight, trace=False):
    B, N_in = 64, 2048
    n_cores = 8
    b_loc = B // n_cores
    key = (N_in, b_loc, n_cores)
    if key not in _CACHE:
        _CACHE[key] = build_module(n_in=N_in, b_loc=b_loc, num_devices=n_cores)
    nc = _CACHE[key]

    x = np.asarray(x, dtype=np.float32)
    weight = np.asarray(weight, dtype=np.float32)
    w2 = host_prep_w(weight, N_in)
    d1, ds = host_prep_deltas()
    ident = np.eye(128, dtype=np.float32).astype(ml_dtypes.bfloat16)
    in_maps = []
    for c in range(n_cores):
        xs = np.ascontiguousarray(x[c * b_loc:(c + 1) * b_loc, :, 0, :])
        in_maps.append({
            "w2": w2,
            "xbd": host_prep_xbd(xs, N_in),
            "d1": d1,
            "ds": ds,
            "ident": ident,
        })
    global LAST_EXEC_NS
    res = run_bass_kernel_spmd(nc, in_maps, core_ids=list(range(n_cores)),
                               trace=trace)
    LAST_EXEC_NS = res.exec_time_ns
    outs = [r["out"].reshape(b_loc, D, J).transpose(0, 2, 1) for r in res.results]
    return np.ascontiguousarray(np.concatenate(outs, axis=0))


# revision 3
# speedup vs baseline: 1.0372x; 1.0199x over previous
"""CapsuleLayer (dynamic routing, ROUTING_ITER=2) Bass/Tile kernel for TRN2 — v2.

Contract: kernel(x, weight) takes FULL inputs
  x:      [64, 2048, 1, 16] f32
  weight: [1, 2048, 32, 16, 16] f32
returns FULL output [64, 32, 16] f32.

Sharding: data-parallel over batch B=64 across 8 cores (8 per core),
weight replicated. Self-contained: hardcodes shapes, imports only
numpy/ml_dtypes/concourse.

v2 changes vs v1:
 - T = sum_d(u*v) reduction moved from a DVE bf16 add-tree to PE
   identity-stationary matmuls accumulating in PSUM (f32).
 - prod/y elementwise multiplies split DVE / GPSIMD(Pool).
 - softmax: exp on Act engine straight out of PSUM; sum/recip/scale DVE.
 - xbd host layout transposed so DMA descriptors are 2KB contiguous.
 - routing pass software-pipelined (prod leads; y trails 2 chunks, s trails
   3) so DVE/Pool/PE all stream without dependency stalls.
 - PE filler/warm matmuls across chunk and pass boundaries keep the tensor
   engine's p-state ramped (idle PE restarts its clock ramp).
 - small first/last W DMA chunks shorten phase-1 ramp-in/out.
"""

from contextlib import ExitStack

import ml_dtypes
import numpy as np

import concourse.bacc as bacc
import concourse.bass as bass
import concourse.mybir as mybir
import concourse.tile as tile
from concourse.bass_utils import run_bass_kernel_spmd

F32 = mybir.dt.float32
BF16 = mybir.dt.bfloat16
AF = mybir.ActivationFunctionType
AX = mybir.AxisListType

EPS = 1e-8
USE_GPSIMD = True
J, D, E = 32, 16, 16
JD = J * D  # 512


def emit_capsule(tc, w2, xbd, d1, ds, ident, out, n_in, b_loc=8):
    """Emit the per-core capsule program.

    DRAM tensors (APs):
      w2    [G, 8, E, JD] bf16  weight, host-permuted to [i, e, d, j], i=g*8+di
      xbd   [128, G, 64]  bf16  block-diag x stationary: [(di,e), g, (b,di')]
      d1    [128, 8] bf16       delta matrix * (1/32)  (s1 accumulation)
      ds    [128, 8] bf16       delta matrix * 1.0     (s2/s3 accumulation)
      ident [128, 128] bf16     identity (PE partition-copy stationary)
      out   [b_loc, JD] f32     squash(s3) output, (j,d) layout

    u layout in SBUF (bf16): partition p = (g%2)*64 + b*8 + di,
    free f = (g//2)*JD + d*J + j, where i = g*8 + di.
    """
    nc = tc.nc
    assert b_loc == 8
    G = n_in // 8
    GH = G // 2          # 128 gh slices of [128, JD]
    GDMA = 16            # groups per W DMA chunk
    CHB = 16             # gh per routing chunk (16*JD = 8K free)
    NCH = GH // CHB      # 8 chunks per pass
    # gh split per chunk between DVE and GPSIMD for prod/y muls
    GH_DVE = 12

    ctx = ExitStack()
    singles = ctx.enter_context(tc.tile_pool(name="singles", bufs=1))
    small = ctx.enter_context(tc.tile_pool(name="small", bufs=1))
    dramp = ctx.enter_context(tc.tile_pool(name="dram_scratch", bufs=1, space="DRAM"))

    u_sb = singles.tile([128, GH * JD], BF16)
    d1_sb = singles.tile([128, 8], BF16)
    ds_sb = singles.tile([128, 8], BF16)
    id_sb = singles.tile([128, 128], BF16)
    nc.sync.dma_start(out=d1_sb, in_=d1)
    nc.sync.dma_start(out=ds_sb, in_=ds)
    nc.sync.dma_start(out=id_sb, in_=ident)
    v_exp = singles.tile([128, JD], BF16)
    V = singles.tile([8, JD], F32)      # running sum of v vectors
    vscr = dramp.tile([8, JD], BF16)    # DRAM bounce buffer for v broadcast
    eps8 = singles.tile([8, 1], F32)
    nc.vector.memset(eps8, EPS)
    warmp = ctx.enter_context(tc.tile_pool(name="warm", bufs=1, space="PSUM"))
    warm_ps = warmp.tile([128, JD], F32)

    def pe_warm(n=24, mov=None):
        # throwaway matmuls that keep the PE clock ramped across the serial
        # squash/v-broadcast boundaries (it would otherwise idle and restart
        # its p-state ramp at the start of the next pass). Passing mov chains
        # them on a boundary-produced tile so they land inside the boundary.
        if mov is None:
            mov = u_sb[:, :JD]
        for _ in range(n):
            nc.tensor.matmul(warm_ps, id_sb, mov, start=True, stop=True)

    # ---------- squash helpers (all on 8 partitions, tiny) ----------
    def squash_j(s_in, v_out):
        # v = squash(s, axis=j):  sq[b,d] = sum_j s^2;  v = s*sq/((1+sq)*sqrt(sq+eps))
        t2 = small.tile([8, JD], F32, tag="sqt2")
        nc.scalar.square(t2, s_in)
        sv = small.tile([8, 4, J], F32, tag="sqv")
        sq, a, t3, w = sv[:, 0, :D], sv[:, 1, :D], sv[:, 2, :D], sv[:, 3, :D]
        nc.vector.reduce_sum(out=sq, in_=t2.rearrange("p (d j) -> p d j", d=D), axis=AX.X)
        nc.scalar.activation(a, sq, AF.Sqrt, bias=eps8)
        nc.vector.tensor_mul(t3, sq, a)
        nc.vector.tensor_add(t3, t3, a)          # a*(1+sq)
        nc.vector.reciprocal(w, t3)
        nc.vector.tensor_mul(w, w, sq)           # sq/((1+sq)a)
        wb = w.unsqueeze(2).broadcast_to([8, D, J])
        nc.vector.tensor_mul(v_out.rearrange("p (d j) -> p d j", d=D),
                             s_in.rearrange("p (d j) -> p d j", d=D), wb)

    def squash_d(s_in, v_out):
        # v = squash(s, axis=d): sq[b,j] = sum_d s^2
        t2 = small.tile([8, JD], F32, tag="sqt2")
        nc.scalar.square(t2, s_in)
        sv = small.tile([8, 4, J], F32, tag="sqv")
        sq, a, t3, w = sv[:, 0, :], sv[:, 1, :], sv[:, 2, :], sv[:, 3, :]
        nc.vector.reduce_sum(out=sq, in_=t2.rearrange("p (d j) -> p j d", d=D), axis=AX.X)
        nc.scalar.activation(a, sq, AF.Sqrt, bias=eps8)
        nc.vector.tensor_mul(t3, sq, a)
        nc.vector.tensor_add(t3, t3, a)
        nc.vector.reciprocal(w, t3)
        nc.vector.tensor_mul(w, w, sq)
        wb = w.unsqueeze(1).broadcast_to([8, D, J])
        nc.vector.tensor_mul(v_out.rearrange("p (d j) -> p d j", d=D),
                             s_in.rearrange("p (d j) -> p d j", d=D), wb)

    def refresh_v_exp():
        vb = small.tile([8, JD], BF16, tag="vb")
        nc.vector.tensor_copy(out=vb, in_=V)
        nc.sync.dma_start(out=vscr, in_=vb)
        src = vscr.unsqueeze(1).broadcast_to([8, 8, JD])
        for g2 in range(2):
            nc.sync.dma_start(out=v_exp[g2 * 64:(g2 + 1) * 64, :], in_=src)
        return vb

    # ---------- phase 1: W pass (u = W @ x), s1 accumulation ----------
    with tc.tile_pool(name="wp", bufs=3) as wp, \
         tc.tile_pool(name="xp", bufs=2) as xp, \
         tc.tile_pool(name="up", bufs=2, space="PSUM") as up, \
         tc.tile_pool(name="sp", bufs=1, space="PSUM") as sp:
        s1_ps = sp.tile([8, JD], F32)
        # first W chunks are small so PE starts ~7us earlier; rest are 16-group
        chunks = [4, 4, 4, 4] + [GDMA] * ((G - 32) // GDMA) + [4, 4, 4, 4]
        g_base = 0
        for csz in chunks:
            g0, g1 = g_base, g_base + csz
            g_base = g1
            wt = wp.tile([128, csz, JD], BF16, tag=f"wt{csz}",
                         bufs=2 if csz < GDMA else None)
            # src: dims (k=(di,e) merged, g, jd)
            wsrc = w2[g0:g1].rearrange("g di e f -> (di e) g f")
            nc.sync.dma_start(out=wt, in_=wsrc)
            xt = xp.tile([128, csz, 64], BF16, tag=f"xt{csz}")
            nc.sync.dma_start(out=xt, in_=xbd[:, g0:g1])
            for gq in range(csz // 4):
                pt = up.tile([128, 2 * JD], F32, tag="upt")
                for idx in range(4):
                    gl = gq * 4 + idx
                    nc.tensor.matmul(
                        pt[(gl % 2) * 64:(gl % 2) * 64 + 64,
                           (idx // 2) * JD:(idx // 2) * JD + JD],
                        xt[:, gl, :], wt[:, gl, :], start=True, stop=True)
                gh0 = g0 // 2 + gq * 2
                dst = u_sb[:, gh0 * JD:(gh0 + 2) * JD]
                if gq % 2 == 0:
                    nc.vector.tensor_copy(out=dst, in_=pt)
                else:
                    nc.scalar.copy(out=dst, in_=pt)
                for gh in (gh0, gh0 + 1):
                    nc.tensor.matmul(s1_ps, d1_sb, u_sb[:, gh * JD:(gh + 1) * JD],
                                     start=(gh == 0), stop=(gh == GH - 1))
        squash_j(s1_ps, V)      # V = v1
        refresh_v_exp()
        pe_warm(22)
        pe_warm(20, mov=v_exp)

    # ---------- routing pass (T = u.V via PE, softmax, s = sum_i c*u) ----------
    def routing_pass(final):
        # software-pipelined: stage A(k) = prod + T matmuls; stage B(k) =
        # softmax + y + s matmuls, issued one chunk behind so DVE/Pool never
        # stall waiting on PE/Act.
        with tc.tile_pool(name="rp", bufs=4) as rp, \
             tc.tile_pool(name="ep", bufs=3) as epool, \
             tc.tile_pool(name="cp", bufs=1) as cp, \
             tc.tile_pool(name="tp", bufs=2, space="PSUM") as tp, \
             tc.tile_pool(name="spp", bufs=1, space="PSUM") as spp:
            s_ps = spp.tile([8, JD], F32)
            tiles = {}

            def stage_prod(k):
                gh0 = k * CHB
                u_ch = u_sb[:, gh0 * JD:(gh0 + CHB) * JD].rearrange(
                    "p (g f) -> p g f", g=CHB)
                vb = v_exp.unsqueeze(1).broadcast_to([128, CHB, JD])
                prod = rp.tile([128, CHB, JD], BF16, tag="prod")
                # prod = u * v_bcast, split DVE(13) / GPSIMD(3) by gh range
                if USE_GPSIMD:
                    nc.vector.tensor_mul(prod[:, :13], u_ch[:, :13], vb[:, :13])
                    nc.gpsimd.tensor_mul(prod[:, 13:], u_ch[:, 13:], vb[:, 13:])
                else:
                    nc.vector.tensor_mul(prod, u_ch, vb)
                tiles[k] = (u_ch, prod)

            def stage_T(k):
                u_ch, prod = tiles[k]
                # T[p, gh, j] = sum_d prod[p, gh, d, j]  on PE via identity
                p4 = prod.rearrange("p g (d j) -> p g d j", d=D)
                t_ps = tp.tile([128, CHB, J], F32, tag="T")
                for d in range(D):
                    nc.tensor.matmul(t_ps, id_sb, p4[:, :, d, :],
                                     start=(d == 0), stop=(d == D - 1))
                # softmax over j (no max subtraction; logits are tiny)
                eT = epool.tile([128, CHB, J], BF16, tag="eT")
                nc.scalar.activation(eT, t_ps, AF.Exp)
                tiles[k] = (u_ch, prod, p4, eT)

            def stage_y(k):
                u_ch, prod, p4, eT = tiles[k]
                u4 = u_ch.rearrange("p g (d j) -> p g d j", d=D)
                se = cp.tile([128, CHB], F32, tag="se")
                nc.vector.reduce_sum(out=se, in_=eT, axis=AX.X)
                r = cp.tile([128, CHB], BF16, tag="r")
                with nc.allow_low_precision(reason="softmax denom, rel err 4e-3 ok"):
                    nc.vector.reciprocal(r, se)
                c = cp.tile([128, CHB, J], BF16, tag="c")
                nc.vector.tensor_mul(c, eT, r.unsqueeze(2).broadcast_to([128, CHB, J]))
                # y = c (broadcast over d) * u — overwrites prod in place,
                # DVE(12) / GPSIMD(4) split
                cb = c.unsqueeze(2).broadcast_to([128, CHB, D, J])
                if USE_GPSIMD:
                    nc.vector.tensor_mul(p4[:, :12], u4[:, :12], cb[:, :12])
                    nc.gpsimd.tensor_mul(p4[:, 12:], u4[:, 12:], cb[:, 12:])
                else:
                    nc.vector.tensor_mul(p4, u4, cb)

            def stage_s(k):
                gh0 = k * CHB
                _, prod, _, _ = tiles.pop(k)
                for q in range(CHB):
                    gh = gh0 + q
                    nc.tensor.matmul(s_ps, ds_sb, prod[:, q],
                                     start=(gh == 0), stop=(gh == GH - 1))

            # PE order per iteration: s(k-3) [deps long ready], filler warm
            # matmuls, then T(k) — backlog covers prod(k) latency so the PE
            # stream (and its p-state ramp) never breaks; y trails by 2
            for k in range(NCH + 3):
                if k < NCH:
                    stage_prod(k)
                if 2 <= k < NCH + 2:
                    stage_y(k - 2)
                if k >= 3:
                    stage_s(k - 3)
                if k < NCH:
                    if k >= 1:
                        pe_warm(4)
                    stage_T(k)
            if not final:
                v2 = small.tile([8, JD], F32, tag="v2")
                squash_j(s_ps, v2)
                nc.vector.tensor_add(V, V, v2)
                refresh_v_exp()
                pe_warm(22)
                pe_warm(20, mov=v_exp)
            else:
                vout = small.tile([8, JD], F32, tag="vout")
                squash_d(s_ps, vout)
                nc.sync.dma_start(out=out, in_=vout)

    routing_pass(final=False)   # iteration 2 (uses V=v1)
    routing_pass(final=True)    # final (uses V=v1+v2)
    ctx.close()


def build_module(n_in=2048, b_loc=8, num_devices=8, enable_asserts=False):
    nc = bacc.Bacc("TRN2", target_bir_lowering=False, debug=False,
                   num_devices=num_devices, enable_asserts=enable_asserts)
    G = n_in // 8
    w2 = nc.dram_tensor("w2", [G, 8, E, JD], BF16, kind="ExternalInput").ap()
    xbd = nc.dram_tensor("xbd", [128, G, 64], BF16, kind="ExternalInput").ap()
    d1 = nc.dram_tensor("d1", [128, 8], BF16, kind="ExternalInput").ap()
    ds = nc.dram_tensor("ds", [128, 8], BF16, kind="ExternalInput").ap()
    ident = nc.dram_tensor("ident", [128, 128], BF16, kind="ExternalInput").ap()
    out = nc.dram_tensor("out", [b_loc, JD], F32, kind="ExternalOutput").ap()
    with tile.TileContext(nc) as tc:
        emit_capsule(tc, w2, xbd, d1, ds, ident, out, n_in=n_in, b_loc=b_loc)
    nc.compile()
    return nc


def host_prep_w(weight, n_in):
    # weight [1, N, J, D, E] -> w2 [G, 8, E, J*D] with free layout (d, j)
    w2 = np.ascontiguousarray(weight[0].transpose(0, 3, 2, 1))  # [N, E, D, J]
    return w2.reshape(n_in // 8, 8, E, JD).astype(ml_dtypes.bfloat16)


def host_prep_xbd(xs, n_in):
    # xs [b_loc, N, E] -> xbd [128, G, 64] block-diagonal stationary,
    # partition-major so each DMA descriptor is one contiguous 2KB run.
    G = n_in // 8
    t = xs.reshape(8, G, 8, E).transpose(1, 2, 3, 0)  # [G, di, e, b]
    xbd = np.zeros((G, 8, E, 8, 8), np.float32)       # [G, di, e, b, di']
    for di in range(8):
        xbd[:, di, :, :, di] = t[:, di]
    xbd = xbd.reshape(G, 128, 64).transpose(1, 0, 2)  # [(di,e), G, (b,di')]
    return np.ascontiguousarray(xbd).astype(ml_dtypes.bfloat16)


def host_prep_deltas():
    p = np.arange(128)
    bofp = (p // 8) % 8
    d1 = np.zeros((128, 8), np.float32)
    ds = np.zeros((128, 8), np.float32)
    d1[p, bofp] = 1.0 / 32.0
    ds[p, bofp] = 1.0
    return d1.astype(ml_dtypes.bfloat16), ds.astype(ml_dtypes.bfloat16)


_CACHE = {}
LAST_EXEC_NS = None


def kernel(x, weight, trace=False):
    B, N_in = 64, 2048
    n_cores = 8
    b_loc = B // n_cores
    key = (N_in, b_loc, n_cores)
    if key not in _CACHE:
        _CACHE[key] = build_module(n_in=N_in, b_loc=b_loc, num_devices=n_cores)
    nc = _CACHE[key]

    x = np.asarray(x, dtype=np.float32)
    weight = np.asarray(weight, dtype=np.float32)
    w2 = host_prep_w(weight, N_in)
    d1, ds = host_prep_deltas()
    ident = np.eye(128, dtype=np.float32).astype(ml_dtypes.bfloat16)
    in_maps = []
    for c in range(n_cores):
        xs = np.ascontiguousarray(x[c * b_loc:(c + 1) * b_loc, :, 0, :])
        in_maps.append({
            "w2": w2,
            "xbd": host_prep_xbd(xs, N_in),
            "d1": d1,
            "ds": ds,
            "ident": ident,
        })
    global LAST_EXEC_NS
    res = run_bass_kernel_spmd(nc, in_maps, core_ids=list(range(n_cores)),
                               trace=trace)
    LAST_EXEC_NS = res.exec_time_ns
    outs = [r["out"].reshape(b_loc, D, J).transpose(0, 2, 1) for r in res.results]
    return np.ascontiguousarray(np.concatenate(outs, axis=0))


# revision 4
# speedup vs baseline: 1.0868x; 1.0478x over previous
"""CapsuleLayer (dynamic routing, ROUTING_ITER=2) Bass/Tile kernel for TRN2 — v2.

Contract: kernel(x, weight) takes FULL inputs
  x:      [64, 2048, 1, 16] f32
  weight: [1, 2048, 32, 16, 16] f32
returns FULL output [64, 32, 16] f32.

Sharding: data-parallel over batch B=64 across 8 cores (8 per core),
weight replicated. Self-contained: hardcodes shapes, imports only
numpy/ml_dtypes/concourse.

v2 changes vs v1:
 - T = sum_d(u*v) reduction moved from a DVE bf16 add-tree to PE
   identity-stationary matmuls accumulating in PSUM (f32).
 - prod/y elementwise multiplies split DVE / GPSIMD(Pool).
 - softmax: exp on Act engine straight out of PSUM; sum/recip/scale DVE.
 - xbd host layout transposed so DMA descriptors are 2KB contiguous.
 - routing pass software-pipelined (prod leads; y trails 2 chunks, s trails
   3) so DVE/Pool/PE all stream without dependency stalls.
 - PE filler/warm matmuls across chunk and pass boundaries keep the tensor
   engine's p-state ramped (idle PE restarts its clock ramp).
 - small first/last W DMA chunks shorten phase-1 ramp-in/out.
"""

from contextlib import ExitStack

import ml_dtypes
import numpy as np

import concourse.bacc as bacc
import concourse.bass as bass
import concourse.mybir as mybir
import concourse.tile as tile
from concourse.bass_utils import run_bass_kernel_spmd

F32 = mybir.dt.float32
BF16 = mybir.dt.bfloat16
AF = mybir.ActivationFunctionType
AX = mybir.AxisListType

EPS = 1e-8
USE_GPSIMD = True
J, D, E = 32, 16, 16
JD = J * D  # 512


def emit_capsule(tc, w2, xbd, d1, ds, ident, out, n_in, b_loc=8):
    """Emit the per-core capsule program.

    DRAM tensors (APs):
      w2    [G, 8, E, JD] bf16  weight, host-permuted to [i, e, d, j], i=g*8+di
      xbd   [128, G, 64]  bf16  block-diag x stationary: [(di,e), g, (b,di')]
      d1    [128, 8] bf16       delta matrix * (1/32)  (s1 accumulation)
      ds    [128, 8] bf16       delta matrix * 1.0     (s2/s3 accumulation)
      ident [128, 128] bf16     identity (PE partition-copy stationary)
      out   [b_loc, JD] f32     squash(s3) output, (j,d) layout

    u layout in SBUF (bf16): partition p = (g%2)*64 + b*8 + di,
    free f = (g//2)*JD + d*J + j, where i = g*8 + di.
    """
    nc = tc.nc
    assert b_loc == 8
    G = n_in // 8
    GH = G // 2          # 128 gh slices of [128, JD]
    GDMA = 16            # groups per W DMA chunk
    CHB = 16             # gh per routing chunk (16*JD = 8K free)
    NCH = GH // CHB      # 8 chunks per pass
    # gh split per chunk between DVE and GPSIMD for prod/y muls
    GH_DVE = 12

    ctx = ExitStack()
    singles = ctx.enter_context(tc.tile_pool(name="singles", bufs=1))
    small = ctx.enter_context(tc.tile_pool(name="small", bufs=1))
    dramp = ctx.enter_context(tc.tile_pool(name="dram_scratch", bufs=1, space="DRAM"))

    u_sb = singles.tile([128, GH * JD], BF16)
    d1_sb = singles.tile([128, 8], BF16)
    ds_sb = singles.tile([128, 8], BF16)
    id_sb = singles.tile([128, 128], BF16)
    nc.sync.dma_start(out=d1_sb, in_=d1)
    nc.sync.dma_start(out=ds_sb, in_=ds)
    nc.sync.dma_start(out=id_sb, in_=ident)
    v_exp = singles.tile([128, JD], BF16)
    V = singles.tile([8, JD], F32)      # running sum of v vectors
    vscr = dramp.tile([8, JD], BF16)    # DRAM bounce buffer for v broadcast
    eps8 = singles.tile([8, 1], F32)
    nc.vector.memset(eps8, EPS)
    warmp = ctx.enter_context(tc.tile_pool(name="warm", bufs=1, space="PSUM"))
    warm_ps = warmp.tile([128, JD], F32)

    def pe_warm(n=24, mov=None):
        # throwaway matmuls that keep the PE clock ramped across the serial
        # squash/v-broadcast boundaries (it would otherwise idle and restart
        # its p-state ramp at the start of the next pass). Passing mov chains
        # them on a boundary-produced tile so they land inside the boundary.
        if mov is None:
            mov = u_sb[:, :JD]
        for _ in range(n):
            nc.tensor.matmul(warm_ps, id_sb, mov, start=True, stop=True)

    # ---------- squash helpers (all on 8 partitions, tiny) ----------
    def squash_j(s_in, v_out):
        # v = squash(s, axis=j):  sq[b,d] = sum_j s^2;  v = s*sq/((1+sq)*sqrt(sq+eps))
        t2 = small.tile([8, JD], F32, tag="sqt2")
        nc.scalar.square(t2, s_in)
        sv = small.tile([8, 4, J], F32, tag="sqv")
        sq, a, t3, w = sv[:, 0, :D], sv[:, 1, :D], sv[:, 2, :D], sv[:, 3, :D]
        nc.vector.reduce_sum(out=sq, in_=t2.rearrange("p (d j) -> p d j", d=D), axis=AX.X)
        nc.scalar.activation(a, sq, AF.Sqrt, bias=eps8)
        nc.vector.tensor_mul(t3, sq, a)
        nc.vector.tensor_add(t3, t3, a)          # a*(1+sq)
        nc.vector.reciprocal(w, t3)
        nc.vector.tensor_mul(w, w, sq)           # sq/((1+sq)a)
        wb = w.unsqueeze(2).broadcast_to([8, D, J])
        nc.vector.tensor_mul(v_out.rearrange("p (d j) -> p d j", d=D),
                             s_in.rearrange("p (d j) -> p d j", d=D), wb)
        return t2

    def squash_d(s_in, v_out):
        # v = squash(s, axis=d): sq[b,j] = sum_d s^2
        t2 = small.tile([8, JD], F32, tag="sqt2")
        nc.scalar.square(t2, s_in)
        sv = small.tile([8, 4, J], F32, tag="sqv")
        sq, a, t3, w = sv[:, 0, :], sv[:, 1, :], sv[:, 2, :], sv[:, 3, :]
        nc.vector.reduce_sum(out=sq, in_=t2.rearrange("p (d j) -> p j d", d=D), axis=AX.X)
        nc.scalar.activation(a, sq, AF.Sqrt, bias=eps8)
        nc.vector.tensor_mul(t3, sq, a)
        nc.vector.tensor_add(t3, t3, a)
        nc.vector.reciprocal(w, t3)
        nc.vector.tensor_mul(w, w, sq)
        wb = w.unsqueeze(1).broadcast_to([8, D, J])
        nc.vector.tensor_mul(v_out.rearrange("p (d j) -> p d j", d=D),
                             s_in.rearrange("p (d j) -> p d j", d=D), wb)

    def refresh_v_exp():
        vb = small.tile([8, JD], BF16, tag="vb")
        nc.vector.tensor_copy(out=vb, in_=V)
        nc.sync.dma_start(out=vscr, in_=vb)
        src = vscr.unsqueeze(1).broadcast_to([8, 8, JD])
        for g2 in range(2):
            nc.sync.dma_start(out=v_exp[g2 * 64:(g2 + 1) * 64, :], in_=src)
        return vb

    # ---------- phase 1: W pass (u = W @ x), s1 accumulation ----------
    with tc.tile_pool(name="wp", bufs=3) as wp, \
         tc.tile_pool(name="xp", bufs=2) as xp, \
         tc.tile_pool(name="up", bufs=3, space="PSUM") as up, \
         tc.tile_pool(name="sp", bufs=1, space="PSUM") as sp:
        s1_ps = sp.tile([8, JD], F32)
        # first W chunks are small so PE starts ~7us earlier; rest are 16-group
        chunks = [4, 4, 4, 4] + [GDMA] * ((G - 32) // GDMA) + [4, 4, 4, 4]
        g_base = 0
        for csz in chunks:
            g0, g1 = g_base, g_base + csz
            g_base = g1
            wt = wp.tile([128, csz, JD], BF16, tag=f"wt{csz}",
                         bufs=2 if csz < GDMA else None)
            # src: dims (k=(di,e) merged, g, jd)
            wsrc = w2[g0:g1].rearrange("g di e f -> (di e) g f")
            nc.sync.dma_start(out=wt, in_=wsrc)
            xt = xp.tile([128, csz, 64], BF16, tag=f"xt{csz}")
            nc.sync.dma_start(out=xt, in_=xbd[:, g0:g1])
            for gq in range(csz // 4):
                pt = up.tile([128, 2 * JD], F32, tag="upt")
                for idx in range(4):
                    gl = gq * 4 + idx
                    nc.tensor.matmul(
                        pt[(gl % 2) * 64:(gl % 2) * 64 + 64,
                           (idx // 2) * JD:(idx // 2) * JD + JD],
                        xt[:, gl, :], wt[:, gl, :], start=True, stop=True)
                gh0 = g0 // 2 + gq * 2
                dst = u_sb[:, gh0 * JD:(gh0 + 2) * JD]
                if gq % 2 == 0:
                    nc.vector.tensor_copy(out=dst, in_=pt)
                else:
                    nc.scalar.copy(out=dst, in_=pt)
                for gh in (gh0, gh0 + 1):
                    nc.tensor.matmul(s1_ps, d1_sb, u_sb[:, gh * JD:(gh + 1) * JD],
                                     start=(gh == 0), stop=(gh == GH - 1))
        squash_j(s1_ps, V)      # V = v1
        refresh_v_exp()
        pe_warm(22)
        pe_warm(20, mov=v_exp)

    # ---------- routing pass (T = u.V via PE, softmax, s = sum_i c*u) ----------
    def routing_pass(final):
        # software-pipelined: stage A(k) = prod + T matmuls; stage B(k) =
        # softmax + y + s matmuls, issued one chunk behind so DVE/Pool never
        # stall waiting on PE/Act.
        with tc.tile_pool(name="rp", bufs=4) as rp, \
             tc.tile_pool(name="ep", bufs=3) as epool, \
             tc.tile_pool(name="cp", bufs=1) as cp, \
             tc.tile_pool(name="tp", bufs=2, space="PSUM") as tp, \
             tc.tile_pool(name="spp", bufs=1, space="PSUM") as spp:
            s_ps = spp.tile([8, JD], F32)
            tiles = {}

            def stage_prod(k):
                gh0 = k * CHB
                u_ch = u_sb[:, gh0 * JD:(gh0 + CHB) * JD].rearrange(
                    "p (g f) -> p g f", g=CHB)
                vb = v_exp.unsqueeze(1).broadcast_to([128, CHB, JD])
                prod = rp.tile([128, CHB, JD], BF16, tag="prod")
                # prod = u * v_bcast, split DVE(13) / GPSIMD(3) by gh range
                if USE_GPSIMD:
                    nc.vector.tensor_mul(prod[:, :13], u_ch[:, :13], vb[:, :13])
                    nc.gpsimd.tensor_mul(prod[:, 13:], u_ch[:, 13:], vb[:, 13:])
                else:
                    nc.vector.tensor_mul(prod, u_ch, vb)
                tiles[k] = (u_ch, prod)

            def stage_T(k):
                u_ch, prod = tiles[k]
                # T[p, gh, j] = sum_d prod[p, gh, d, j]  on PE via identity
                p4 = prod.rearrange("p g (d j) -> p g d j", d=D)
                t_ps = tp.tile([128, CHB, J], F32, tag="T")
                for d in range(D):
                    nc.tensor.matmul(t_ps, id_sb, p4[:, :, d, :],
                                     start=(d == 0), stop=(d == D - 1))
                # softmax over j (no max subtraction; logits are tiny)
                eT = epool.tile([128, CHB, J], BF16, tag="eT")
                nc.scalar.activation(eT, t_ps, AF.Exp)
                tiles[k] = (u_ch, prod, p4, eT)

            def stage_y(k):
                u_ch, prod, p4, eT = tiles[k]
                u4 = u_ch.rearrange("p g (d j) -> p g d j", d=D)
                se = cp.tile([128, CHB], F32, tag="se")
                nc.vector.reduce_sum(out=se, in_=eT, axis=AX.X)
                r = cp.tile([128, CHB], BF16, tag="r")
                with nc.allow_low_precision(reason="softmax denom, rel err 4e-3 ok"):
                    nc.vector.reciprocal(r, se)
                c = cp.tile([128, CHB, J], BF16, tag="c")
                nc.vector.tensor_mul(c, eT, r.unsqueeze(2).broadcast_to([128, CHB, J]))
                # y = c (broadcast over d) * u — overwrites prod in place,
                # DVE(12) / GPSIMD(4) split
                cb = c.unsqueeze(2).broadcast_to([128, CHB, D, J])
                if USE_GPSIMD:
                    nc.vector.tensor_mul(p4[:, :12], u4[:, :12], cb[:, :12])
                    nc.gpsimd.tensor_mul(p4[:, 12:], u4[:, 12:], cb[:, 12:])
                else:
                    nc.vector.tensor_mul(p4, u4, cb)

            def stage_s(k):
                gh0 = k * CHB
                _, prod, _, _ = tiles.pop(k)
                for q in range(CHB):
                    gh = gh0 + q
                    nc.tensor.matmul(s_ps, ds_sb, prod[:, q],
                                     start=(gh == 0), stop=(gh == GH - 1))

            # PE order per iteration: s(k-3) [deps long ready], filler warm
            # matmuls, then T(k) — backlog covers prod(k) latency so the PE
            # stream (and its p-state ramp) never breaks; y trails by 2
            for k in range(NCH + 3):
                if k < NCH:
                    stage_prod(k)
                if 2 <= k < NCH + 2:
                    stage_y(k - 2)
                if k >= 3:
                    stage_s(k - 3)
                if k < NCH:
                    if k >= 1:
                        pe_warm(3)
                    stage_T(k)
            if not final:
                v2 = small.tile([8, JD], F32, tag="v2")
                squash_j(s_ps, v2)
                nc.vector.tensor_add(V, V, v2)
                refresh_v_exp()
                pe_warm(22)
                pe_warm(20, mov=v_exp)
            else:
                vout = small.tile([8, JD], F32, tag="vout")
                squash_d(s_ps, vout)
                nc.sync.dma_start(out=out, in_=vout)

    routing_pass(final=False)   # iteration 2 (uses V=v1)
    routing_pass(final=True)    # final (uses V=v1+v2)
    ctx.close()


def build_module(n_in=2048, b_loc=8, num_devices=8, enable_asserts=False):
    nc = bacc.Bacc("TRN2", target_bir_lowering=False, debug=False,
                   num_devices=num_devices, enable_asserts=enable_asserts)
    G = n_in // 8
    w2 = nc.dram_tensor("w2", [G, 8, E, JD], BF16, kind="ExternalInput").ap()
    xbd = nc.dram_tensor("xbd", [128, G, 64], BF16, kind="ExternalInput").ap()
    d1 = nc.dram_tensor("d1", [128, 8], BF16, kind="ExternalInput").ap()
    ds = nc.dram_tensor("ds", [128, 8], BF16, kind="ExternalInput").ap()
    ident = nc.dram_tensor("ident", [128, 128], BF16, kind="ExternalInput").ap()
    out = nc.dram_tensor("out", [b_loc, JD], F32, kind="ExternalOutput").ap()
    with tile.TileContext(nc) as tc:
        emit_capsule(tc, w2, xbd, d1, ds, ident, out, n_in=n_in, b_loc=b_loc)
    nc.compile()
    return nc


def host_prep_w(weight, n_in):
    # weight [1, N, J, D, E] -> w2 [G, 8, E, J*D] with free layout (d, j)
    w2 = np.ascontiguousarray(weight[0].transpose(0, 3, 2, 1))  # [N, E, D, J]
    return w2.reshape(n_in // 8, 8, E, JD).astype(ml_dtypes.bfloat16)


def host_prep_xbd(xs, n_in):
    # xs [b_loc, N, E] -> xbd [128, G, 64] block-diagonal stationary,
    # partition-major so each DMA descriptor is one contiguous 2KB run.
    G = n_in // 8
    t = xs.reshape(8, G, 8, E).transpose(1, 2, 3, 0)  # [G, di, e, b]
    xbd = np.zeros((G, 8, E, 8, 8), np.float32)       # [G, di, e, b, di']
    for di in range(8):
        xbd[:, di, :, :, di] = t[:, di]
    xbd = xbd.reshape(G, 128, 64).transpose(1, 0, 2)  # [(di,e), G, (b,di')]
    return np.ascontiguousarray(xbd).astype(ml_dtypes.bfloat16)


def host_prep_deltas():
    p = np.arange(128)
    bofp = (p // 8) % 8
    d1 = np.zeros((128, 8), np.float32)
    ds = np.zeros((128, 8), np.float32)
    d1[p, bofp] = 1.0 / 32.0
    ds[p, bofp] = 1.0
    return d1.astype(ml_dtypes.bfloat16), ds.astype(ml_dtypes.bfloat16)


_CACHE = {}
LAST_EXEC_NS = None


def kernel(x, weight, trace=False):
    B, N_in = 64, 2048
    n_cores = 8
    b_loc = B // n_cores
    key = (N_in, b_loc, n_cores)
    if key not in _CACHE:
        _CACHE[key] = build_module(n_in=N_in, b_loc=b_loc, num_devices=n_cores)
    nc = _CACHE[key]

    x = np.asarray(x, dtype=np.float32)
    weight = np.asarray(weight, dtype=np.float32)
    w2 = host_prep_w(weight, N_in)
    d1, ds = host_prep_deltas()
    ident = np.eye(128, dtype=np.float32).astype(ml_dtypes.bfloat16)
    in_maps = []
    for c in range(n_cores):
        xs = np.ascontiguousarray(x[c * b_loc:(c + 1) * b_loc, :, 0, :])
        in_maps.append({
            "w2": w2,
            "xbd": host_prep_xbd(xs, N_in),
            "d1": d1,
            "ds": ds,
            "ident": ident,
        })
    global LAST_EXEC_NS
    res = run_bass_kernel_spmd(nc, in_maps, core_ids=list(range(n_cores)),
                               trace=trace)
    LAST_EXEC_NS = res.exec_time_ns
    outs = [r["out"].reshape(b_loc, D, J).transpose(0, 2, 1) for r in res.results]
    return np.ascontiguousarray(np.concatenate(outs, axis=0))


# revision 5
# speedup vs baseline: 1.0888x; 1.0019x over previous
"""CapsuleLayer (dynamic routing, ROUTING_ITER=2) Bass/Tile kernel for TRN2 — v2.

Contract: kernel(x, weight) takes FULL inputs
  x:      [64, 2048, 1, 16] f32
  weight: [1, 2048, 32, 16, 16] f32
returns FULL output [64, 32, 16] f32.

Sharding: data-parallel over batch B=64 across 8 cores (8 per core),
weight replicated. Self-contained: hardcodes shapes, imports only
numpy/ml_dtypes/concourse.

v2 changes vs v1:
 - T = sum_d(u*v) reduction moved from a DVE bf16 add-tree to PE
   identity-stationary matmuls accumulating in PSUM (f32).
 - prod/y elementwise multiplies split DVE / GPSIMD(Pool).
 - softmax: exp on Act engine straight out of PSUM; sum/recip/scale DVE.
 - xbd host layout transposed so DMA descriptors are 2KB contiguous.
 - routing pass software-pipelined (prod leads; y trails 2 chunks, s trails
   3) so DVE/Pool/PE all stream without dependency stalls.
 - PE filler/warm matmuls across chunk and pass boundaries keep the tensor
   engine's p-state ramped (idle PE restarts its clock ramp).
 - small first/last W DMA chunks shorten phase-1 ramp-in/out.
"""

from contextlib import ExitStack

import ml_dtypes
import numpy as np

import concourse.bacc as bacc
import concourse.bass as bass
import concourse.mybir as mybir
import concourse.tile as tile
from concourse.bass_utils import run_bass_kernel_spmd

F32 = mybir.dt.float32
BF16 = mybir.dt.bfloat16
AF = mybir.ActivationFunctionType
AX = mybir.AxisListType

EPS = 1e-8
USE_GPSIMD = True
J, D, E = 32, 16, 16
JD = J * D  # 512


def emit_capsule(tc, w2, xbd, d1, ds, ident, sel, out, n_in, b_loc=8):
    """Emit the per-core capsule program.

    DRAM tensors (APs):
      w2    [G, 8, E, JD] bf16  weight, host-permuted to [i, e, d, j], i=g*8+di
      xbd   [128, G, 64]  bf16  block-diag x stationary: [(di,e), g, (b,di')]
      d1    [128, 8] bf16       delta matrix * (1/32)  (s1 accumulation)
      ds    [128, 8] bf16       delta matrix * 1.0     (s2/s3 accumulation)
      ident [128, 128] bf16     identity (PE partition-copy stationary)
      out   [b_loc, JD] f32     squash(s3) output, (j,d) layout

    u layout in SBUF (bf16): partition p = (g%2)*64 + b*8 + di,
    free f = (g//2)*JD + d*J + j, where i = g*8 + di.
    """
    nc = tc.nc
    assert b_loc == 8
    G = n_in // 8
    GH = G // 2          # 128 gh slices of [128, JD]
    GDMA = 16            # groups per W DMA chunk
    CHB = 16             # gh per routing chunk (16*JD = 8K free)
    NCH = GH // CHB      # 8 chunks per pass
    # gh split per chunk between DVE and GPSIMD for prod/y muls
    GH_DVE = 12

    ctx = ExitStack()
    singles = ctx.enter_context(tc.tile_pool(name="singles", bufs=1))
    small = ctx.enter_context(tc.tile_pool(name="small", bufs=1))

    u_sb = singles.tile([128, GH * JD], BF16)
    d1_sb = singles.tile([128, 8], BF16)
    ds_sb = singles.tile([128, 8], BF16)
    id_sb = singles.tile([128, 128], BF16)
    sel_sb = singles.tile([128, 128], F32)
    v_exp = singles.tile([128, JD], BF16)
    # running sum of v vectors; rows 8..127 stay zero so it can be the
    # moving operand of the PE partition-broadcast below
    V = singles.tile([128, JD], F32)
    eps8 = singles.tile([8, 1], F32)
    nc.vector.memset(eps8, EPS)
    nc.vector.memset(V, 0.0)
    warmp = ctx.enter_context(tc.tile_pool(name="warm", bufs=1, space="PSUM"))
    warm_ps = warmp.tile([128, JD], F32)

    def load_consts():
        # issued after the first W chunk's DMA so they don't delay PE start
        nc.sync.dma_start(out=d1_sb, in_=d1)
        nc.sync.dma_start(out=ds_sb, in_=ds)
        nc.sync.dma_start(out=id_sb, in_=ident)
        nc.sync.dma_start(out=sel_sb, in_=sel)

    def pe_warm(n=24, mov=None):
        # throwaway matmuls that keep the PE clock ramped across the serial
        # squash/v-broadcast boundaries (it would otherwise idle and restart
        # its p-state ramp at the start of the next pass). Passing mov chains
        # them on a boundary-produced tile so they land inside the boundary.
        if mov is None:
            mov = u_sb[:, :JD]
        for _ in range(n):
            nc.tensor.matmul(warm_ps, id_sb, mov, start=True, stop=True)

    # ---------- squash helpers (all on 8 partitions, tiny) ----------
    def squash_j(s_in, v_out):
        # v = squash(s, axis=j):  sq[b,d] = sum_j s^2;  v = s*sq/((1+sq)*sqrt(sq+eps))
        t2 = small.tile([8, JD], F32, tag="sqt2")
        nc.scalar.square(t2, s_in)
        sv = small.tile([8, 4, J], F32, tag="sqv")
        sq, a, t3, w = sv[:, 0, :D], sv[:, 1, :D], sv[:, 2, :D], sv[:, 3, :D]
        nc.vector.reduce_sum(out=sq, in_=t2.rearrange("p (d j) -> p d j", d=D), axis=AX.X)
        # sqrt(sq+eps) = exp(0.5*ln(sq+eps)): keeps Act on a single function
        # table (ln/exp/square/copy), avoiding 1.3us table reloads per swap
        nc.scalar.activation(t3, sq, AF.Ln, bias=eps8)
        nc.scalar.activation(a, t3, AF.Exp, scale=0.5)
        nc.vector.tensor_mul(t3, sq, a)
        nc.vector.tensor_add(t3, t3, a)          # a*(1+sq)
        nc.vector.reciprocal(w, t3)
        nc.vector.tensor_mul(w, w, sq)           # sq/((1+sq)a)
        wb = w.unsqueeze(2).broadcast_to([8, D, J])
        nc.vector.tensor_mul(v_out.rearrange("p (d j) -> p d j", d=D),
                             s_in.rearrange("p (d j) -> p d j", d=D), wb)
        return t2

    def squash_d(s_in, v_out):
        # v = squash(s, axis=d): sq[b,j] = sum_d s^2
        t2 = small.tile([8, JD], F32, tag="sqt2")
        nc.scalar.square(t2, s_in)
        sv = small.tile([8, 4, J], F32, tag="sqv")
        sq, a, t3, w = sv[:, 0, :], sv[:, 1, :], sv[:, 2, :], sv[:, 3, :]
        nc.vector.reduce_sum(out=sq, in_=t2.rearrange("p (d j) -> p j d", d=D), axis=AX.X)
        nc.scalar.activation(t3, sq, AF.Ln, bias=eps8)
        nc.scalar.activation(a, t3, AF.Exp, scale=0.5)
        nc.vector.tensor_mul(t3, sq, a)
        nc.vector.tensor_add(t3, t3, a)
        nc.vector.reciprocal(w, t3)
        nc.vector.tensor_mul(w, w, sq)
        wb = w.unsqueeze(1).broadcast_to([8, D, J])
        nc.vector.tensor_mul(v_out.rearrange("p (d j) -> p d j", d=D),
                             s_in.rearrange("p (d j) -> p d j", d=D), wb)

    def refresh_v_exp():
        # partition-broadcast V[b] -> v_exp[p] (b = (p//8)%8) on PE: one
        # selector matmul into PSUM + one Act copy to bf16 SBUF. Much shorter
        # than the previous DRAM-bounce DMA chain (~1.5us vs ~5us).
        nc.tensor.matmul(warm_ps, sel_sb, V, start=True, stop=True)
        nc.scalar.copy(out=v_exp, in_=warm_ps)

    # ---------- phase 1: W pass (u = W @ x), s1 accumulation ----------
    with tc.tile_pool(name="wp", bufs=3) as wp, \
         tc.tile_pool(name="xp", bufs=2) as xp, \
         tc.tile_pool(name="up", bufs=3, space="PSUM") as up, \
         tc.tile_pool(name="sp", bufs=1, space="PSUM") as sp:
        s1_ps = sp.tile([8, JD], F32)
        # first W chunks are small so PE starts ~7us earlier; rest are 16-group
        chunks = [4, 4, 4, 4] + [GDMA] * ((G - 32) // GDMA) + [4, 4, 4, 4]
        g_base = 0
        for csz in chunks:
            g0, g1 = g_base, g_base + csz
            g_base = g1
            wt = wp.tile([128, csz, JD], BF16, tag=f"wt{csz}",
                         bufs=2 if csz < GDMA else None)
            # src: dims (k=(di,e) merged, g, jd)
            wsrc = w2[g0:g1].rearrange("g di e f -> (di e) g f")
            nc.sync.dma_start(out=wt, in_=wsrc)
            xt = xp.tile([128, csz, 64], BF16, tag=f"xt{csz}")
            nc.sync.dma_start(out=xt, in_=xbd[:, g0:g1])
            if g0 == 0:
                load_consts()
            for gq in range(csz // 4):
                pt = up.tile([128, 2 * JD], F32, tag="upt")
                for idx in range(4):
                    gl = gq * 4 + idx
                    nc.tensor.matmul(
                        pt[(gl % 2) * 64:(gl % 2) * 64 + 64,
                           (idx // 2) * JD:(idx // 2) * JD + JD],
                        xt[:, gl, :], wt[:, gl, :], start=True, stop=True)
                gh0 = g0 // 2 + gq * 2
                dst = u_sb[:, gh0 * JD:(gh0 + 2) * JD]
                if gq % 2 == 0:
                    nc.vector.tensor_copy(out=dst, in_=pt)
                else:
                    nc.scalar.copy(out=dst, in_=pt)
                for gh in (gh0, gh0 + 1):
                    nc.tensor.matmul(s1_ps, d1_sb, u_sb[:, gh * JD:(gh + 1) * JD],
                                     start=(gh == 0), stop=(gh == GH - 1))
        squash_j(s1_ps, V[:8])      # V = v1
        refresh_v_exp()
        pe_warm(16)
        pe_warm(10, mov=v_exp)

    # ---------- routing pass (T = u.V via PE, softmax, s = sum_i c*u) ----------
    def routing_pass(final):
        # software-pipelined: stage A(k) = prod + T matmuls; stage B(k) =
        # softmax + y + s matmuls, issued one chunk behind so DVE/Pool never
        # stall waiting on PE/Act.
        with tc.tile_pool(name="rp", bufs=4) as rp, \
             tc.tile_pool(name="ep", bufs=3) as epool, \
             tc.tile_pool(name="cp", bufs=1) as cp, \
             tc.tile_pool(name="tp", bufs=2, space="PSUM") as tp, \
             tc.tile_pool(name="spp", bufs=1, space="PSUM") as spp:
            s_ps = spp.tile([8, JD], F32)
            tiles = {}

            def stage_prod(k):
                gh0 = k * CHB
                u_ch = u_sb[:, gh0 * JD:(gh0 + CHB) * JD].rearrange(
                    "p (g f) -> p g f", g=CHB)
                vb = v_exp.unsqueeze(1).broadcast_to([128, CHB, JD])
                prod = rp.tile([128, CHB, JD], BF16, tag="prod")
                # prod = u * v_bcast, split DVE(13) / GPSIMD(3) by gh range
                if USE_GPSIMD:
                    nc.vector.tensor_mul(prod[:, :13], u_ch[:, :13], vb[:, :13])
                    nc.gpsimd.tensor_mul(prod[:, 13:], u_ch[:, 13:], vb[:, 13:])
                else:
                    nc.vector.tensor_mul(prod, u_ch, vb)
                tiles[k] = (u_ch, prod)

            def stage_T(k):
                u_ch, prod = tiles[k]
                # T[p, gh, j] = sum_d prod[p, gh, d, j]  on PE via identity
                p4 = prod.rearrange("p g (d j) -> p g d j", d=D)
                t_ps = tp.tile([128, CHB, J], F32, tag="T")
                for d in range(D):
                    nc.tensor.matmul(t_ps, id_sb, p4[:, :, d, :],
                                     start=(d == 0), stop=(d == D - 1))
                # softmax over j (no max subtraction; logits are tiny)
                eT = epool.tile([128, CHB, J], BF16, tag="eT")
                nc.scalar.activation(eT, t_ps, AF.Exp)
                tiles[k] = (u_ch, prod, p4, eT)

            def stage_y(k):
                u_ch, prod, p4, eT = tiles[k]
                u4 = u_ch.rearrange("p g (d j) -> p g d j", d=D)
                se = cp.tile([128, CHB], F32, tag="se")
                nc.vector.reduce_sum(out=se, in_=eT, axis=AX.X)
                r = cp.tile([128, CHB], BF16, tag="r")
                with nc.allow_low_precision(reason="softmax denom, rel err 4e-3 ok"):
                    nc.vector.reciprocal(r, se)
                # fold the softmax denominator into the s-matmul stationary:
                # rd[p, gh, b'] = r[p, gh] * delta_{b(p), b'}. The y mul then
                # uses exp(T) directly, saving a [128, CHB*J] multiply.
                rd = cp.tile([128, CHB, 8], BF16, tag="rd", bufs=4)
                nc.vector.tensor_mul(rd, r.unsqueeze(2).broadcast_to([128, CHB, 8]),
                                     ds_sb.unsqueeze(1).broadcast_to([128, CHB, 8]))
                # y = exp(T) (broadcast over d) * u — overwrites prod in place,
                # DVE(12) / GPSIMD(4) split
                cb = eT.unsqueeze(2).broadcast_to([128, CHB, D, J])
                if USE_GPSIMD:
                    nc.vector.tensor_mul(p4[:, :12], u4[:, :12], cb[:, :12])
                    nc.gpsimd.tensor_mul(p4[:, 12:], u4[:, 12:], cb[:, 12:])
                else:
                    nc.vector.tensor_mul(p4, u4, cb)
                tiles[k] = (u_ch, prod, p4, eT, rd)

            def stage_s(k):
                gh0 = k * CHB
                _, prod, _, _, rd = tiles.pop(k)
                for q in range(CHB):
                    gh = gh0 + q
                    nc.tensor.matmul(s_ps, rd[:, q], prod[:, q],
                                     start=(gh == 0), stop=(gh == GH - 1))

            # PE order per iteration: s(k-3) [deps long ready], filler warm
            # matmuls, then T(k) — backlog covers prod(k) latency so the PE
            # stream (and its p-state ramp) never breaks; y trails by 2
            for k in range(NCH + 3):
                if k < NCH:
                    stage_prod(k)
                if 2 <= k < NCH + 2:
                    stage_y(k - 2)
                if k >= 3:
                    stage_s(k - 3)
                if k < NCH:
                    if k >= 1:
                        pe_warm(3)
                    stage_T(k)
            if not final:
                v2 = small.tile([8, JD], F32, tag="v2")
                squash_j(s_ps, v2)
                nc.vector.tensor_add(V[:8], V[:8], v2)
                refresh_v_exp()
                pe_warm(16)
                pe_warm(10, mov=v_exp)
            else:
                vout = small.tile([8, JD], F32, tag="vout")
                squash_d(s_ps, vout)
                nc.sync.dma_start(out=out, in_=vout)

    routing_pass(final=False)   # iteration 2 (uses V=v1)
    routing_pass(final=True)    # final (uses V=v1+v2)
    ctx.close()


def build_module(n_in=2048, b_loc=8, num_devices=8, enable_asserts=False):
    nc = bacc.Bacc("TRN2", target_bir_lowering=False, debug=False,
                   num_devices=num_devices, enable_asserts=enable_asserts)
    G = n_in // 8
    w2 = nc.dram_tensor("w2", [G, 8, E, JD], BF16, kind="ExternalInput").ap()
    xbd = nc.dram_tensor("xbd", [128, G, 64], BF16, kind="ExternalInput").ap()
    d1 = nc.dram_tensor("d1", [128, 8], BF16, kind="ExternalInput").ap()
    ds = nc.dram_tensor("ds", [128, 8], BF16, kind="ExternalInput").ap()
    ident = nc.dram_tensor("ident", [128, 128], BF16, kind="ExternalInput").ap()
    sel = nc.dram_tensor("sel", [128, 128], F32, kind="ExternalInput").ap()
    out = nc.dram_tensor("out", [b_loc, JD], F32, kind="ExternalOutput").ap()
    with tile.TileContext(nc) as tc:
        emit_capsule(tc, w2, xbd, d1, ds, ident, sel, out, n_in=n_in, b_loc=b_loc)
    nc.compile()
    return nc


def host_prep_w(weight, n_in):
    # weight [1, N, J, D, E] -> w2 [G, 8, E, J*D] with free layout (d, j)
    w2 = np.ascontiguousarray(weight[0].transpose(0, 3, 2, 1))  # [N, E, D, J]
    return w2.reshape(n_in // 8, 8, E, JD).astype(ml_dtypes.bfloat16)


def host_prep_xbd(xs, n_in):
    # xs [b_loc, N, E] -> xbd [128, G, 64] block-diagonal stationary,
    # partition-major so each DMA descriptor is one contiguous 2KB run.
    G = n_in // 8
    t = xs.reshape(8, G, 8, E).transpose(1, 2, 3, 0)  # [G, di, e, b]
    xbd = np.zeros((G, 8, E, 8, 8), np.float32)       # [G, di, e, b, di']
    for di in range(8):
        xbd[:, di, :, :, di] = t[:, di]
    xbd = xbd.reshape(G, 128, 64).transpose(1, 0, 2)  # [(di,e), G, (b,di')]
    return np.ascontiguousarray(xbd).astype(ml_dtypes.bfloat16)


def host_prep_deltas():
    p = np.arange(128)
    bofp = (p // 8) % 8
    d1 = np.zeros((128, 8), np.float32)
    ds = np.zeros((128, 8), np.float32)
    d1[p, bofp] = 1.0 / 32.0
    ds[p, bofp] = 1.0
    return d1.astype(ml_dtypes.bfloat16), ds.astype(ml_dtypes.bfloat16)


_CACHE = {}
LAST_EXEC_NS = None


def kernel(x, weight, trace=False):
    B, N_in = 64, 2048
    n_cores = 8
    b_loc = B // n_cores
    key = (N_in, b_loc, n_cores)
    if key not in _CACHE:
        _CACHE[key] = build_module(n_in=N_in, b_loc=b_loc, num_devices=n_cores)
    nc = _CACHE[key]

    x = np.asarray(x, dtype=np.float32)
    weight = np.asarray(weight, dtype=np.float32)
    w2 = host_prep_w(weight, N_in)
    d1, ds = host_prep_deltas()
    ident = np.eye(128, dtype=np.float32).astype(ml_dtypes.bfloat16)
    selm = np.zeros((128, 128), np.float32)
    selm[(np.arange(128) // 8) % 8, np.arange(128)] = 1.0
    in_maps = []
    for c in range(n_cores):
        xs = np.ascontiguousarray(x[c * b_loc:(c + 1) * b_loc, :, 0, :])
        in_maps.append({
            "w2": w2,
            "xbd": host_prep_xbd(xs, N_in),
            "d1": d1,
            "ds": ds,
            "ident": ident,
            "sel": selm,
        })
    global LAST_EXEC_NS
    res = run_bass_kernel_spmd(nc, in_maps, core_ids=list(range(n_cores)),
                               trace=trace)
    LAST_EXEC_NS = res.exec_time_ns
    outs = [r["out"].reshape(b_loc, D, J).transpose(0, 2, 1) for r in res.results]
    return np.ascontiguousarray(np.concatenate(outs, axis=0))
